# revision 15
# baseline (speedup 1.0000x reference)
"""Trainium2 Bass kernel for nn_CentralAttention1 (sparse_attention).

Self-contained: takes the FULL (unsharded) inputs as numpy arrays, shards
batch 8-ways across the NeuronCores (data parallel; each core gets
batch/8 rows of each of the 3 agents), runs a single SPMD Bass program,
and gathers the full output.

Dataflow on device is "transposed": activations live as
[features on partitions, batch on the free dim].  The conv stack is
expressed as banded-Toeplitz PE matmuls streaming directly into an fc1
PSUM accumulation; BatchNorm train-mode statistics are a [60,2]
AllReduce; the 3-agent masked softmax reduces to a sigmoid; the two
Q-MLP heads run per agent-block.  All matmuls use the float32r PE path
(1 cycle/row at free>=256, ~1e-4 relative precision).
"""

import os
import numpy as np

import concourse.bass as bass
import concourse.bacc as bacc
import concourse.tile as tile
from concourse import mybir
from concourse.bass_utils import run_bass_kernel_spmd

# ---- problem sizes (hardcoded per the task spec) ----
NAG, B, H, HEADS, AD = 3, 4096, 128, 8, 16
STATE, ACTD, SCAN, OUTF, HID = 48, 2, 256, 10, 256
EPS = 1e-5
NCORES = 8
BL = B // NCORES            # 512 rows per agent per core
R = NAG * BL                # 1536 rows per core
NB = BL                     # free-dim block = one agent block
P2 = 250                    # conv2 output positions
QT = 63                     # conv tiles of 4 positions (252 = 63*4)
NTOT = NAG * B

F32 = mybir.dt.float32
F32R = mybir.dt.float32r
AX = mybir.AxisListType.X
AF = mybir.ActivationFunctionType
OP = mybir.AluOpType


def _t1_parts(q):
    """conv1 tile q -> list of (t1 stack index, scan block index)."""
    if q <= 30:
        return [(q, 0)]
    if q == 31:
        return [(31, 0), (32, 1)]
    return [(q - 32, 1)]


def build_program():
    nc = bacc.Bacc(num_devices=NCORES)

    scan_t = nc.dram_tensor("scan_t", [SCAN, R], F32R, kind="ExternalInput")
    obs_t = nc.dram_tensor("obs_t", [STATE, R], F32R, kind="ExternalInput")
    acts_t = nc.dram_tensor("acts_t", [ACTD, R], F32R, kind="ExternalInput")
    t1_d = nc.dram_tensor("t1", [128, 33, 128], F32R, kind="ExternalInput")
    t2_d = nc.dram_tensor("t2", [128, 320], F32R, kind="ExternalInput")
    fc1w_d = nc.dram_tensor("fc1w", [128, QT, 256], F32R, kind="ExternalInput")
    fc2w_d = nc.dram_tensor("fc2w", [128, 2, OUTF], F32R, kind="ExternalInput")
    encw_d = nc.dram_tensor("encw", [128, 128], F32R, kind="ExternalInput")
    attw_d = nc.dram_tensor("attw", [128, 5, 128], F32R, kind="ExternalInput")
    hsum_d = nc.dram_tensor("hsum", [128, 16], F32R, kind="ExternalInput")
    hbc_d = nc.dram_tensor("hbc", [8, 128], F32R, kind="ExternalInput")
    mlpw_d = nc.dram_tensor("mlpw", [128, 2, 1152], F32R, kind="ExternalInput")
    bias_d = nc.dram_tensor("bias", [128, 20], F32, kind="ExternalInput")
    zpad_d = nc.dram_tensor("zpad", [30, R], F32R, kind="ExternalInput")
    out_d = nc.dram_tensor("out", [2, R], F32, kind="ExternalOutput")
    dbg_d = nc.dram_tensor("dbg", [128, 10, NB], F32, kind="ExternalOutput")
    dbg2_d = nc.dram_tensor("dbg2", [8, NB], F32, kind="ExternalOutput")
    dbg3_d = nc.dram_tensor("dbg3", [128, 6], F32, kind="ExternalOutput")

    with tile.TileContext(nc) as tc:
        with (
            tc.tile_pool(name="dram", bufs=1, space="DRAM") as dram,
            tc.tile_pool(name="cst", bufs=1) as cst,
            tc.tile_pool(name="ypool", bufs=3) as ypool,
            tc.tile_pool(name="opool", bufs=3) as opool,
            tc.tile_pool(name="xpool", bufs=4) as xpool,
            tc.tile_pool(name="wp2", bufs=2) as wp2,
            tc.tile_pool(name="wp4", bufs=4) as wp4,
        ):
            # ---- weight / input DMAs (program order ~ priority) ----
            s0 = cst.tile([128, R], F32R, tag="s0")
            nc.sync.dma_start(out=s0, in_=scan_t[0:128, :])
            s1 = cst.tile([128, R], F32R, tag="s1")
            nc.sync.dma_start(out=s1, in_=scan_t[128:256, :])
            t1c = []
            for k in range(4):
                n = 9 if k < 3 else 6
                t = cst.tile([128, n, 128], F32R, tag=f"t1c{k}")
                nc.sync.dma_start(out=t, in_=t1_d[:, 9 * k:9 * k + n, :])
                t1c.append(t)
            t2sb = cst.tile([128, 320], F32R, tag="t2")
            nc.sync.dma_start(out=t2sb, in_=t2_d[:])
            biasb = cst.tile([128, 20], F32, tag="bias")
            nc.sync.dma_start(out=biasb, in_=bias_d[:])
            # BN feature rows in 32-aligned groups: obs 0:48, feats 64:74,
            # acts 96:98; everything else stays zero.
            inps = cst.tile([128, R], F32R, tag="inps")
            nc.sync.dma_start(out=inps[0:STATE, :], in_=obs_t[:])
            nc.sync.dma_start(out=inps[96:96 + ACTD, :], in_=acts_t[:])
            nc.sync.dma_start(out=inps[48:64, :], in_=zpad_d[0:16, :])
            nc.sync.dma_start(out=inps[74:96, :], in_=zpad_d[0:22, :])
            nc.sync.dma_start(out=inps[98:128, :], in_=zpad_d[0:30, :])
            fc1c = []
            for k in range(8):
                n = 8 if k < 7 else 7
                t = cst.tile([128, n, 256], F32R, tag=f"fc1c{k}")
                nc.sync.dma_start(out=t, in_=fc1w_d[:, 8 * k:8 * k + n, :])
                fc1c.append(t)
            fc2w = cst.tile([128, 2, OUTF], F32R, tag="fc2w")
            nc.sync.dma_start(out=fc2w, in_=fc2w_d[:])
            encw = cst.tile([128, 128], F32R, tag="encw")
            nc.sync.dma_start(out=encw, in_=encw_d[:])
            attw = cst.tile([128, 5, 128], F32R, tag="attw")
            nc.sync.dma_start(out=attw, in_=attw_d[:])
            hsum = cst.tile([128, 16], F32R, tag="hsum")
            nc.sync.dma_start(out=hsum, in_=hsum_d[:])
            hbc = cst.tile([8, 128], F32R, tag="hbc")
            nc.sync.dma_start(out=hbc, in_=hbc_d[:])
            mlpw = cst.tile([128, 2, 1152], F32R, tag="mlpw")
            nc.sync.dma_start(out=mlpw, in_=mlpw_d[:])

            saT = cst.tile([128, NAG, NB], F32R, tag="saT")
            keysT = cst.tile([128, NAG, NB], F32R, tag="keysT")
            valsT = cst.tile([128, NAG, NB], F32R, tag="valsT")
            outq1 = cst.tile([1, R], F32, tag="outq1")
            outq2 = cst.tile([1, R], F32, tag="outq2")

            def evict_relu(dst, src_ps, bias_ap, use_act):
                if use_act:
                    nc.scalar.activation(dst, src_ps, AF.Relu, bias=bias_ap)
                else:
                    nc.vector.tensor_scalar(
                        out=dst, in0=src_ps, scalar1=bias_ap, scalar2=0.0,
                        op0=OP.add, op1=OP.max)

            # =========== conv stream (pre-BatchNorm), per agent block ========
            with (
                tc.tile_pool(name="ps_c1", bufs=2, space="PSUM") as ps_c1,
                tc.tile_pool(name="ps_c2", bufs=2, space="PSUM") as ps_c2,
                tc.tile_pool(name="ps_fc", bufs=2, space="PSUM") as ps_fc,
                tc.tile_pool(name="ps_f2", bufs=1, space="PSUM") as ps_f2,
            ):
                for g in range(NAG):
                    col = bass.ts(g, NB)
                    fc_ps = [ps_fc.tile([128, NB], F32, tag="pfc",
                                        name=f"pfc{g}_{m}")
                             for m in range(2)]
                    y_tiles = {}

                    def conv2_and_fc1(t):
                        po = ps_c2.tile([128, NB], F32, tag="po")
                        if t < QT - 1:
                            rows = 128
                            nc.tensor.matmul(po, t2sb[:, 0:128], y_tiles[t][:],
                                             start=True, stop=False)
                            nc.tensor.matmul(po, t2sb[:, 128:256],
                                             y_tiles[t + 1][:],
                                             start=False, stop=True)
                            bcol = 1
                        else:
                            rows = 64
                            nc.tensor.matmul(po[0:64, :], t2sb[:, 256:320],
                                             y_tiles[t][:],
                                             start=True, stop=True)
                            bcol = 17
                        o2 = opool.tile([128, NB], F32R, tag="o2")
                        evict_relu(o2[0:rows, :], po[0:rows, :],
                                   biasb[0:rows, bcol:bcol + 1], t % 2 == 0)
                        for m in range(2):
                            lhs = fc1c[t // 8][0:rows, t % 8,
                                              128 * m:128 * m + 128]
                            nc.tensor.matmul(
                                fc_ps[m], lhs, o2[0:rows, :],
                                start=(t == 0), stop=(t == QT - 1),
                                skip_group_check=True)
                        del y_tiles[t]

                    for q in range(QT):
                        py = ps_c1.tile([128, NB], F32, tag="py")
                        parts = _t1_parts(q)
                        for i, (idx, sb_) in enumerate(parts):
                            src = (s0 if sb_ == 0 else s1)[:, col]
                            nc.tensor.matmul(py, t1c[idx // 9][:, idx % 9, :],
                                             src, start=(i == 0),
                                             stop=(i == len(parts) - 1))
                        yq = ypool.tile([128, NB], F32R, tag="y")
                        evict_relu(yq[:], py[:], biasb[:, 0:1], q % 2 == 1)
                        y_tiles[q] = yq
                        if q >= 1:
                            conv2_and_fc1(q - 1)
                    conv2_and_fc1(QT - 1)

                    # fc1 eviction (+relu+fc_b1), fc2, feats -> inps[50:60]
                    x3 = []
                    for m in range(2):
                        xm = xpool.tile([128, NB], F32R, tag="x3")
                        nc.scalar.activation(xm[:], fc_ps[m][:], AF.Relu,
                                             bias=biasb[:, 2 + m:3 + m])
                        x3.append(xm)
                    pf = ps_f2.tile([OUTF, NB], F32, tag="pfeat")
                    for kb in range(2):
                        nc.tensor.matmul(pf, fc2w[:, kb, :], x3[kb][:],
                                         start=(kb == 0), stop=(kb == 1))
                    nc.scalar.activation(inps[64:64 + OUTF, col], pf[:],
                                         AF.Identity,
                                         bias=biasb[0:OUTF, 4:5])

            # ================= BatchNorm statistics + AllReduce ==============
            stats = cst.tile([128, 2], F32, tag="stats")
            nc.vector.reduce_sum(stats[:, 0:1], inps[:, :], axis=AX)
            sq = cst.tile([128, R], F32, tag="sq")
            nc.vector.tensor_mul(sq[:], inps[:, :], inps[:, :])
            nc.vector.reduce_sum(stats[:, 1:2], sq[:], axis=AX)
            cc_in = dram.tile([128, 2], F32, tag="cc_in")
            cc_out = dram.tile([128, 2], F32, tag="cc_out",
                               addr_space="Shared")
            nc.gpsimd.dma_start(out=cc_in[:], in_=stats[:])
            nc.gpsimd.collective_compute(
                "AllReduce", OP.add,
                replica_groups=[list(range(NCORES))],
                ins=[cc_in.opt()], outs=[cc_out.opt()])
            gst = cst.tile([128, 2], F32, tag="gst")
            nc.gpsimd.dma_start(out=gst[:], in_=cc_out[:])
            mu = cst.tile([128, 1], F32, tag="mu")
            nc.scalar.activation(mu[:], gst[:, 0:1], AF.Copy, scale=1.0 / NTOT)
            musq = cst.tile([128, 1], F32, tag="musq")
            nc.scalar.activation(musq[:], mu[:], AF.Square)
            var = cst.tile([128, 1], F32, tag="var")
            nc.scalar.activation(var[:], gst[:, 1:2], AF.Copy, scale=1.0 / NTOT)
            nc.vector.tensor_sub(var[:], var[:], musq[:])
            nc.sync.dma_start(out=dbg3_d[:, 0:2], in_=stats[:])
            nc.sync.dma_start(out=dbg3_d[:, 2:4], in_=gst[:])
            nc.sync.dma_start(out=dbg3_d[:, 4:5], in_=mu[:])
            sd = cst.tile([128, 1], F32, tag="sd")
            nc.scalar.activation(sd[:], var[:], AF.Sqrt,
                                 bias=biasb[:, 18:19])
            rstd = cst.tile([128, 1], F32, tag="rstd")
            nc.vector.reciprocal(rstd[:], sd[:])

            nc.sync.dma_start(out=dbg3_d[:, 5:6], in_=rstd[:])
            # ======================= post-BN network =========================
            with (
                tc.tile_pool(name="ps_p", bufs=3, space="PSUM") as ps_p,
                tc.tile_pool(name="ps_l", bufs=2, space="PSUM") as ps_l,
                tc.tile_pool(name="ps_o", bufs=2, space="PSUM") as ps_o,
            ):
                def leaky_from_psum(dst, src_ps, bias_ap):
                    tmp = wp2.tile([128, NB], F32, tag="tmp")
                    nc.scalar.activation(tmp[:], src_ps, AF.Identity,
                                         bias=bias_ap)
                    nc.vector.scalar_tensor_tensor(
                        out=dst, in0=tmp[:], scalar=0.01, in1=tmp[:],
                        op0=OP.mult, op1=OP.max)

                for g in range(NAG):
                    col = bass.ts(g, NB)
                    nc.vector.tensor_scalar(
                        out=inps[:, col], in0=inps[:, col],
                        scalar1=mu[:], scalar2=rstd[:],
                        op0=OP.subtract, op1=OP.mult)
                    pe_ = ps_p.tile([128, NB], F32, tag="pp")
                    nc.tensor.matmul(pe_, encw[:], inps[:, col],
                                     start=True, stop=True)
                    leaky_from_psum(saT[:, g, :], pe_[:], biasb[:, 5:6])
                    pk = ps_p.tile([128, NB], F32, tag="pp")
                    nc.tensor.matmul(pk, attw[:, 0, :], saT[:, g, :],
                                     start=True, stop=True)
                    nc.scalar.copy(keysT[:, g, :], pk[:])
                    pv = ps_p.tile([128, NB], F32, tag="pp")
                    nc.tensor.matmul(pv, attw[:, 1, :], saT[:, g, :],
                                     start=True, stop=True)
                    leaky_from_psum(valsT[:, g, :], pv[:], biasb[:, 6:7])

                for g in range(NAG):
                    col = bass.ts(g, NB)
                    oa, ob = [o for o in range(NAG) if o != g]
                    pq_ = ps_p.tile([128, NB], F32, tag="pp")
                    nc.tensor.matmul(pq_, attw[:, 2 + g, :], saT[:, g, :],
                                     start=True, stop=True)
                    qT = wp2.tile([128, NB], F32R, tag="qt")
                    nc.scalar.copy(qT[:], pq_[:])
                    # logit difference (l_a - l_b) accumulated in one PSUM
                    pl = ps_l.tile([8, NB], F32, tag="pl")
                    for o, hcols, st in ((oa, slice(0, 8), True),
                                         (ob, slice(8, 16), False)):
                        prod = wp2.tile([128, NB], F32R, tag="prod")
                        nc.vector.tensor_mul(prod[:], qT[:], keysT[:, o, :])
                        nc.tensor.matmul(pl, hsum[:, hcols], prod[:],
                                         start=st, stop=not st)
                    # wa = sigmoid((l_a - l_b)/4); other = vb + wa*(va - vb)
                    wa = wp2.tile([8, NB], F32R, tag="wa")
                    nc.scalar.activation(wa[:], pl[:], AF.Sigmoid, scale=0.25)
                    pw = ps_p.tile([128, NB], F32, tag="pp")
                    nc.tensor.matmul(pw, hbc[:], wa[:], start=True, stop=True)
                    dv = wp2.tile([128, NB], F32, tag="dv")
                    nc.vector.tensor_sub(dv[:], valsT[:, oa, :],
                                         valsT[:, ob, :])
                    m1 = wp2.tile([128, NB], F32, tag="m1")
                    nc.vector.tensor_mul(m1[:], pw[:], dv[:])
                    oth = wp2.tile([128, NB], F32R, tag="oth")
                    nc.vector.tensor_add(oth[:], m1[:], valsT[:, ob, :])

                    if g == 0:
                        nc.sync.dma_start(out=dbg_d[:, 0, :],
                                          in_=saT[:, 0, :].bitcast(F32))
                        nc.sync.dma_start(out=dbg_d[:, 1, :],
                                          in_=oth[:].bitcast(F32))
                    emb = (saT[:, g, :], oth[:])
                    for net in range(2):
                        h_prev = emb
                        for layer in range(2):
                            coff = 512 * layer
                            h_new = []
                            for mb in range(2):
                                pm = ps_p.tile([128, NB], F32, tag="pp")
                                for kb in range(2):
                                    c0 = coff + kb * 256 + mb * 128
                                    nc.tensor.matmul(
                                        pm, mlpw[:, net, c0:c0 + 128],
                                        h_prev[kb], start=(kb == 0),
                                        stop=(kb == 1))
                                hm = wp4.tile([128, NB], F32R, tag="h")
                                bcol = (7 if net == 0 else 12) + 2 * layer + mb
                                evict_relu(hm[:], pm[:],
                                           biasb[:, bcol:bcol + 1], mb == 0)
                                h_new.append(hm)
                            if g == 0:
                                for mb in range(2):
                                    nc.sync.dma_start(
                                        out=dbg_d[:, 2 + 4 * net
                                                  + 2 * layer + mb, :],
                                        in_=h_new[mb][:].bitcast(F32))
                            h_prev = h_new
                        po_ = ps_o.tile([1, NB], F32, tag="pout")
                        for kb in range(2):
                            nc.tensor.matmul(
                                po_,
                                mlpw[:, net, 1024 + 64 * kb:1025 + 64 * kb],
                                h_prev[kb][:], start=(kb == 0), stop=(kb == 1))
                        bcol = 11 if net == 0 else 16
                        if g == 0:
                            dt_ = wp2.tile([1, NB], F32, tag="dbg2t",
                                           name=f"dbg2t{net}")
                            nc.scalar.copy(dt_[:], po_[:])
                            nc.sync.dma_start(out=dbg2_d[net:net + 1, :],
                                              in_=dt_[:])
                        dst = (outq1 if net == 0 else outq2)[0:1, col]
                        nc.scalar.activation(dst, po_[:], AF.Identity,
                                             bias=biasb[0:1, bcol:bcol + 1])
            nc.sync.dma_start(out=out_d[0:1, :], in_=outq1[:])
            nc.sync.dma_start(out=out_d[1:2, :], in_=outq2[:])
    return nc


# ======================= host-side preparation ===========================

def _prep_shared(i):
    f32 = np.float32
    w1 = np.asarray(i["conv_w1"], f32)[:, 0, :]           # [32, 5]
    w2 = np.asarray(i["conv_w2"], f32)                    # [32, 32, 3]
    fw1 = np.asarray(i["fc_w1"], f32)                     # [256, 8000]
    fw2 = np.asarray(i["fc_w2"], f32)                     # [10, 256]
    encw_ = np.asarray(i["enc_w"], f32)                   # [128, 60]
    Wk = np.asarray(i["Wk"], f32)
    Wv = np.asarray(i["Wv"], f32)
    Wq = np.asarray(i["Wq"], f32)

    t1 = np.zeros((128, 33, 128), f32)
    for idx in range(32):
        r0 = 4 * idx if idx < 31 else 124
        for dp in range(4):
            for j in range(5):
                r = r0 + dp + j
                if r < 128:
                    t1[r, idx, dp::4] = w1[:, j]
    for dp in range(4):
        for r in range(4):
            j = r + 4 - dp
            if 0 <= j < 5:
                t1[r, 32, dp::4] = w1[:, j]

    t2 = np.zeros((128, 320), f32)
    for dp in range(4):
        for j in range(3):
            e = dp + j
            for cp in range(32):
                if e < 4:
                    t2[4 * cp + e, dp:128:4] = w2[:, cp, j]
                else:
                    t2[4 * cp + (e - 4), 128 + dp:256:4] = w2[:, cp, j]
    for dp in range(2):
        for j in range(3):
            e = dp + j
            for cp in range(32):
                t2[4 * cp + e, 256 + dp:320:2] = w2[:, cp, j]

    fc1w = np.zeros((128, QT, 256), f32)
    for q in range(QT - 1):
        for dp in range(4):
            p = 4 * q + dp
            fc1w[dp::4, q, :] = fw1[:, p::P2].T
    for dp in range(2):
        fc1w[dp:64:2, QT - 1, :] = fw1[:, (248 + dp)::P2].T

    fc2w = np.zeros((128, 2, OUTF), f32)
    for kb in range(2):
        fc2w[:, kb, :] = fw2[:, 128 * kb:128 * kb + 128].T

    encw_full = np.zeros((128, 128), f32)
    encw_full[0:STATE, :] = encw_.T[0:STATE, :]            # obs rows
    encw_full[64:64 + OUTF, :] = encw_.T[50:60, :]         # feats rows
    encw_full[96:96 + ACTD, :] = encw_.T[48:50, :]         # acts rows

    attw = np.zeros((128, 5, 128), f32)
    attw[:, 0, :] = Wk.reshape(128, H).T
    attw[:, 1, :] = Wv.reshape(128, H).T
    for g in range(NAG):
        attw[:, 2 + g, :] = Wq[g].reshape(128, H).T

    hs = np.kron(np.eye(8, dtype=f32), np.ones((16, 1), f32))  # [128, 8]
    hsum = np.concatenate([hs, -hs], axis=1)                   # [128, 16]
    hbc = np.ascontiguousarray(hs.T)                           # [8, 128]

    mlpw = np.zeros((128, 2, 1152), f32)
    for net, pre in enumerate(("q1", "q2")):
        mw1 = np.asarray(i[pre + "_w1"], f32)
        mw2 = np.asarray(i[pre + "_w2"], f32)
        mw3 = np.asarray(i[pre + "_w3"], f32)
        for kb in range(2):
            mlpw[:, net, kb * 256:(kb + 1) * 256] = \
                mw1[:, 128 * kb:128 * kb + 128].T
            mlpw[:, net, 512 + kb * 256:512 + (kb + 1) * 256] = \
                mw2[:, 128 * kb:128 * kb + 128].T
            mlpw[:, net, 1024 + 64 * kb] = mw3[0, 128 * kb:128 * kb + 128]

    bias = np.zeros((128, 20), f32)
    bias[:, 0] = np.repeat(np.asarray(i["conv_b1"], f32), 4)
    bias[:, 1] = np.repeat(np.asarray(i["conv_b2"], f32), 4)
    bias[:, 2] = np.asarray(i["fc_b1"], f32)[0:128]
    bias[:, 3] = np.asarray(i["fc_b1"], f32)[128:256]
    bias[0:OUTF, 4] = np.asarray(i["fc_b2"], f32)
    bias[:, 5] = np.asarray(i["enc_b"], f32)
    bias[:, 6] = np.asarray(i["bv"], f32).reshape(128)
    for net, pre in enumerate(("q1", "q2")):
        b1 = np.asarray(i[pre + "_b1"], f32)
        b2 = np.asarray(i[pre + "_b2"], f32)
        b3 = np.asarray(i[pre + "_b3"], f32)
        c0 = 7 if net == 0 else 12
        bias[:, c0] = b1[0:128]
        bias[:, c0 + 1] = b1[128:256]
        bias[:, c0 + 2] = b2[0:128]
        bias[:, c0 + 3] = b2[128:256]
        bias[0, 11 if net == 0 else 16] = b3[0]
    bias[0:64, 17] = np.repeat(np.asarray(i["conv_b2"], f32), 2)
    bias[:, 18] = EPS

    return {
        "t1": t1, "t2": t2, "fc1w": fc1w, "fc2w": fc2w,
        "encw": encw_full, "attw": attw,
        "hsum": hsum, "hbc": hbc, "mlpw": mlpw, "bias": bias,
    }


def _shard(arr, c):
    out = np.empty((R, arr.shape[1]), np.float32)
    for g in range(NAG):
        out[g * BL:(g + 1) * BL] = arr[g * B + c * BL: g * B + (c + 1) * BL]
    return np.ascontiguousarray(out.T)


_CACHE = {}


def _get_prog():
    if "nc" not in _CACHE:
        nc = build_program()
        nc.finalize()
        _CACHE["nc"] = nc
    return _CACHE["nc"]


def _make_in_maps(inputs):
    shared = _prep_shared(inputs)
    obs = np.asarray(inputs["obs"], np.float32)
    acts = np.asarray(inputs["acts"], np.float32)
    scan = np.asarray(inputs["scan"], np.float32)
    in_maps = []
    for c in range(NCORES):
        m = dict(shared)
        m["zpad"] = np.zeros((30, R), np.float32)
        m["scan_t"] = _shard(scan, c)
        m["obs_t"] = _shard(obs, c)
        m["acts_t"] = _shard(acts, c)
        in_maps.append(m)
    return in_maps


def _gather(results):
    q = np.empty((2, NAG, B), np.float32)
    for c, r in enumerate(results):
        o = np.asarray(r["out"]).reshape(2, NAG, BL)
        q[:, :, c * BL:(c + 1) * BL] = o
    q1 = np.ascontiguousarray(q[0].reshape(NTOT, 1))
    q2 = np.ascontiguousarray(q[1].reshape(NTOT, 1))
    return q1, q2


def kernel(**inputs):
    nc = _get_prog()
    in_maps = _make_in_maps(inputs)
    if os.environ.get("KERNEL_BACKEND") == "sim":
        from concourse import bass_interp
        sim = bass_interp.MultiCoreSim(nc, NCORES)
        for c in range(NCORES):
            for k, v in in_maps[c].items():
                sim.cores[c].tensor(k)[:] = v
        sim.simulate()
        results = [{"out": np.array(sim.cores[c].tensor("out"))}
                   for c in range(NCORES)]
        return _gather(results)
    res = run_bass_kernel_spmd(nc, in_maps, list(range(NCORES)))
    return _gather(res.results)


def kernel_profiled(**inputs):
    """Like kernel() but NTFF-traced; returns ((q1, q2), exec_time_ns)."""
    _install_ntff_hook()
    nc = _get_prog()
    in_maps = _make_in_maps(inputs)
    res = run_bass_kernel_spmd(nc, in_maps, list(range(NCORES)), trace=True)
    return _gather(res.results), res.exec_time_ns


def _install_ntff_hook():
    """Provide antenv.axon_hooks (absent in this image) and register the
    ctypes NTFF profile hook against libaxon_pjrt.so."""
    import sys
    import types
    import ctypes
    import contextlib

    if "antenv.axon_hooks" not in sys.modules:
        mod = types.ModuleType("antenv.axon_hooks")
        mod._hook = None
        mod.set_axon_ntff_profile_hook = lambda h: setattr(mod, "_hook", h)
        mod.get_axon_ntff_profile_hook = lambda: mod._hook
        sys.modules["antenv.axon_hooks"] = mod
        import antenv
        antenv.axon_hooks = mod
    mod = sys.modules["antenv.axon_hooks"]
    if mod.get_axon_ntff_profile_hook() is not None:
        return
    so_path = "/opt/axon/libaxon_pjrt.so"
    lib = ctypes.CDLL(so_path)
    if not hasattr(lib, "axon_start_nrt_profile"):
        return
    lib.axon_start_nrt_profile.argtypes = [
        ctypes.POINTER(ctypes.c_int64), ctypes.c_size_t]
    lib.axon_start_nrt_profile.restype = ctypes.c_int64
    lib.axon_stop_nrt_profile.argtypes = [ctypes.c_char_p]
    lib.axon_stop_nrt_profile.restype = ctypes.c_int64

    @contextlib.contextmanager
    def _hook(output_dir, device_ids):
        import jax
        jax.devices()
        if device_ids:
            ids = (ctypes.c_int64 * len(device_ids))(*device_ids)
            rc = lib.axon_start_nrt_profile(ids, len(device_ids))
        else:
            rc = lib.axon_start_nrt_profile(None, 0)
        if rc != 0:
            raise RuntimeError(f"axon_start_nrt_profile rc={rc}")
        try:
            yield
        finally:
            n = lib.axon_stop_nrt_profile(str(output_dir).encode())
            if n < 0:
                raise RuntimeError(f"axon_stop_nrt_profile rc={n}")

    mod.set_axon_ntff_profile_hook(_hook)


# revision 16
# speedup vs baseline: 1.0822x; 1.0822x over previous
"""Trainium2 Bass kernel for nn_CentralAttention1 (sparse_attention).

Self-contained: takes the FULL (unsharded) inputs as numpy arrays, shards
batch 8-ways across the NeuronCores (data parallel; each core gets
batch/8 rows of each of the 3 agents), runs a single SPMD Bass program,
and gathers the full output.

Dataflow on device is "transposed": activations live as
[features on partitions, batch on the free dim].  The conv stack is
expressed as banded-Toeplitz PE matmuls streaming directly into an fc1
PSUM accumulation; BatchNorm train-mode statistics are a [60,2]
AllReduce; the 3-agent masked softmax reduces to a sigmoid; the two
Q-MLP heads run per agent-block.  All matmuls use the float32r PE path
(1 cycle/row at free>=256, ~1e-4 relative precision).
"""

import os
import numpy as np

import concourse.bass as bass
import concourse.bacc as bacc
import concourse.tile as tile
from concourse import mybir
from concourse.bass_utils import run_bass_kernel_spmd

# ---- problem sizes (hardcoded per the task spec) ----
NAG, B, H, HEADS, AD = 3, 4096, 128, 8, 16
STATE, ACTD, SCAN, OUTF, HID = 48, 2, 256, 10, 256
EPS = 1e-5
NCORES = 8
BL = B // NCORES            # 512 rows per agent per core
R = NAG * BL                # 1536 rows per core
NB = BL                     # free-dim block = one agent block
P2 = 250                    # conv2 output positions
QT = 63                     # conv tiles of 4 positions (252 = 63*4)
NTOT = NAG * B

F32 = mybir.dt.float32
F32R = mybir.dt.float32r
AX = mybir.AxisListType.X
AF = mybir.ActivationFunctionType
OP = mybir.AluOpType


def _t1_parts(q):
    """conv1 tile q -> list of (t1 stack index, scan block index)."""
    if q <= 30:
        return [(q, 0)]
    if q == 31:
        return [(31, 0), (32, 1)]
    return [(q - 32, 1)]


def build_program():
    nc = bacc.Bacc(num_devices=NCORES)

    scan_t = nc.dram_tensor("scan_t", [SCAN, R], F32R, kind="ExternalInput")
    obs_t = nc.dram_tensor("obs_t", [STATE, R], F32R, kind="ExternalInput")
    acts_t = nc.dram_tensor("acts_t", [ACTD, R], F32R, kind="ExternalInput")
    t1_d = nc.dram_tensor("t1", [128, 33, 128], F32R, kind="ExternalInput")
    t2_d = nc.dram_tensor("t2", [128, 320], F32R, kind="ExternalInput")
    fc1w_d = nc.dram_tensor("fc1w", [128, QT, 256], F32R, kind="ExternalInput")
    fc2w_d = nc.dram_tensor("fc2w", [128, 2, 16], F32R, kind="ExternalInput")
    encw_d = nc.dram_tensor("encw", [128, 128], F32R, kind="ExternalInput")
    attw_d = nc.dram_tensor("attw", [128, 5, 128], F32R, kind="ExternalInput")
    hsum_d = nc.dram_tensor("hsum", [128, 16], F32R, kind="ExternalInput")
    hbc_d = nc.dram_tensor("hbc", [8, 128], F32R, kind="ExternalInput")
    mlpw_d = nc.dram_tensor("mlpw", [128, 2, 1152], F32R, kind="ExternalInput")
    bias_d = nc.dram_tensor("bias", [128, 20], F32, kind="ExternalInput")
    zpad_d = nc.dram_tensor("zpad", [30, R], F32R, kind="ExternalInput")
    out_d = nc.dram_tensor("out", [2, R], F32, kind="ExternalOutput")

    with tile.TileContext(nc) as tc:
        with (
            tc.tile_pool(name="dram", bufs=1, space="DRAM") as dram,
            tc.tile_pool(name="cst", bufs=1) as cst,
            tc.tile_pool(name="ypool", bufs=4) as ypool,
            tc.tile_pool(name="opool", bufs=3) as opool,
            tc.tile_pool(name="xpool", bufs=4) as xpool,
            tc.tile_pool(name="wp2", bufs=2) as wp2,
            tc.tile_pool(name="wp4", bufs=4) as wp4,
        ):
            # ---- weight / input DMAs (program order ~ priority) ----
            s0 = cst.tile([128, R], F32R, tag="s0")
            nc.sync.dma_start(out=s0, in_=scan_t[0:128, :])
            s1 = cst.tile([128, R], F32R, tag="s1")
            nc.sync.dma_start(out=s1, in_=scan_t[128:256, :])
            t1c = []
            for k in range(4):
                n = 9 if k < 3 else 6
                t = cst.tile([128, n, 128], F32R, tag=f"t1c{k}")
                nc.sync.dma_start(out=t, in_=t1_d[:, 9 * k:9 * k + n, :])
                t1c.append(t)
            t2sb = cst.tile([128, 320], F32R, tag="t2")
            nc.sync.dma_start(out=t2sb, in_=t2_d[:])
            biasb = cst.tile([128, 20], F32, tag="bias")
            nc.sync.dma_start(out=biasb, in_=bias_d[:])
            # BN feature rows in 32-aligned groups: obs 0:48, feats 64:74,
            # acts 96:98; everything else stays zero.
            inps = cst.tile([128, R], F32R, tag="inps")
            nc.sync.dma_start(out=inps[0:STATE, :], in_=obs_t[:])
            nc.sync.dma_start(out=inps[96:96 + ACTD, :], in_=acts_t[:])
            nc.sync.dma_start(out=inps[48:64, :], in_=zpad_d[0:16, :])
            nc.sync.dma_start(out=inps[74:96, :], in_=zpad_d[0:22, :])
            nc.sync.dma_start(out=inps[98:128, :], in_=zpad_d[0:30, :])
            fc1c = []
            for k in range(8):
                n = 8 if k < 7 else 7
                t = cst.tile([128, n, 256], F32R, tag=f"fc1c{k}")
                nc.sync.dma_start(out=t, in_=fc1w_d[:, 8 * k:8 * k + n, :])
                fc1c.append(t)
            fc2w = cst.tile([128, 2, 16], F32R, tag="fc2w")
            nc.sync.dma_start(out=fc2w, in_=fc2w_d[:])
            encw = cst.tile([128, 128], F32R, tag="encw")
            nc.sync.dma_start(out=encw, in_=encw_d[:])
            attw = cst.tile([128, 5, 128], F32R, tag="attw")
            nc.sync.dma_start(out=attw, in_=attw_d[:])
            hsum = cst.tile([128, 16], F32R, tag="hsum")
            nc.sync.dma_start(out=hsum, in_=hsum_d[:])
            hbc = cst.tile([8, 128], F32R, tag="hbc")
            nc.sync.dma_start(out=hbc, in_=hbc_d[:])
            mlpw = cst.tile([128, 2, 1152], F32R, tag="mlpw")
            nc.sync.dma_start(out=mlpw, in_=mlpw_d[:])

            saT = cst.tile([128, NAG, NB], F32R, tag="saT")
            keysT = cst.tile([128, NAG, NB], F32R, tag="keysT")
            valsT = cst.tile([128, NAG, NB], F32R, tag="valsT")
            outq1 = cst.tile([1, R], F32, tag="outq1")
            outq2 = cst.tile([1, R], F32, tag="outq2")

            def evict_relu(dst, src_ps, bias_ap, use_act):
                if use_act:
                    nc.scalar.activation(dst, src_ps, AF.Relu, bias=bias_ap)
                else:
                    nc.vector.tensor_scalar(
                        out=dst, in0=src_ps, scalar1=bias_ap, scalar2=0.0,
                        op0=OP.add, op1=OP.max)

            # =========== conv stream (pre-BatchNorm), per agent block ========
            with (
                tc.tile_pool(name="ps_c1", bufs=2, space="PSUM") as ps_c1,
                tc.tile_pool(name="ps_c2", bufs=2, space="PSUM") as ps_c2,
                tc.tile_pool(name="ps_fc", bufs=2, space="PSUM") as ps_fc,
                tc.tile_pool(name="ps_f2", bufs=1, space="PSUM") as ps_f2,
            ):
                for g in range(NAG):
                    scope = ctx_scope = nc.named_scope(f"conv{g}")
                    ctx_scope.__enter__()
                    col = bass.ts(g, NB)
                    fc_ps = [ps_fc.tile([128, NB], F32, tag="pfc",
                                        name=f"pfc{g}_{m}")
                             for m in range(2)]
                    y_tiles = {}

                    def conv2_and_fc1(t):
                        po = ps_c2.tile([128, NB], F32, tag="po")
                        if t < QT - 1:
                            rows = 128
                            nc.tensor.matmul(po, t2sb[:, 0:128], y_tiles[t][:],
                                             start=True, stop=False)
                            nc.tensor.matmul(po, t2sb[:, 128:256],
                                             y_tiles[t + 1][:],
                                             start=False, stop=True)
                            bcol = 1
                        else:
                            rows = 64
                            nc.tensor.matmul(po[0:64, :], t2sb[:, 256:320],
                                             y_tiles[t][:],
                                             start=True, stop=True)
                            bcol = 17
                        o2 = opool.tile([128, NB], F32R, tag="o2")
                        evict_relu(o2[0:rows, :], po[0:rows, :],
                                   biasb[0:rows, bcol:bcol + 1], t % 2 == 0)
                        for m in range(2):
                            lhs = fc1c[t // 8][0:rows, t % 8,
                                              128 * m:128 * m + 128]
                            nc.tensor.matmul(
                                fc_ps[m], lhs, o2[0:rows, :],
                                start=(t == 0), stop=(t == QT - 1),
                                skip_group_check=True)
                        del y_tiles[t]

                    for q in range(QT):
                        py = ps_c1.tile([128, NB], F32, tag="py")
                        parts = _t1_parts(q)
                        for i, (idx, sb_) in enumerate(parts):
                            src = (s0 if sb_ == 0 else s1)[:, col]
                            nc.tensor.matmul(py, t1c[idx // 9][:, idx % 9, :],
                                             src, start=(i == 0),
                                             stop=(i == len(parts) - 1))
                        yq = ypool.tile([128, NB], F32R, tag="y")
                        evict_relu(yq[:], py[:], biasb[:, 0:1], q % 2 == 1)
                        y_tiles[q] = yq
                        if q >= 1:
                            conv2_and_fc1(q - 1)
                    conv2_and_fc1(QT - 1)

                    # fc1 eviction (+relu+fc_b1), fc2, feats -> inps[50:60]
                    x3 = []
                    for m in range(2):
                        xm = xpool.tile([128, NB], F32R, tag="x3")
                        nc.scalar.activation(xm[:], fc_ps[m][:], AF.Relu,
                                             bias=biasb[:, 2 + m:3 + m])
                        x3.append(xm)
                    pf = ps_f2.tile([OUTF, NB], F32, tag="pfeat")
                    for kb in range(2):
                        nc.tensor.matmul(pf, fc2w[:, kb, 0:OUTF], x3[kb][:],
                                         start=(kb == 0), stop=(kb == 1))
                    nc.scalar.activation(inps[64:64 + OUTF, col], pf[:],
                                         AF.Identity,
                                         bias=biasb[0:OUTF, 4:5])
                    ctx_scope.__exit__(None, None, None)

            # ================= BatchNorm statistics + AllReduce ==============
            bn_scope = nc.named_scope("bn")
            bn_scope.__enter__()
            stats = cst.tile([128, 2], F32, tag="stats")
            nc.vector.reduce_sum(stats[:, 0:1], inps[:, :], axis=AX)
            sq = cst.tile([128, R], F32, tag="sq")
            nc.vector.tensor_mul(sq[:], inps[:, :], inps[:, :])
            nc.vector.reduce_sum(stats[:, 1:2], sq[:], axis=AX)
            cc_in = dram.tile([128, 2], F32, tag="cc_in")
            cc_out = dram.tile([128, 2], F32, tag="cc_out",
                               addr_space="Shared")
            nc.gpsimd.dma_start(out=cc_in[:], in_=stats[:])
            nc.gpsimd.collective_compute(
                "AllReduce", OP.add,
                replica_groups=[list(range(NCORES))],
                ins=[cc_in.opt()], outs=[cc_out.opt()])
            gst = cst.tile([128, 2], F32, tag="gst")
            nc.gpsimd.dma_start(out=gst[:], in_=cc_out[:])
            mu = cst.tile([128, 1], F32, tag="mu")
            nc.scalar.activation(mu[:], gst[:, 0:1], AF.Copy, scale=1.0 / NTOT)
            musq = cst.tile([128, 1], F32, tag="musq")
            nc.scalar.activation(musq[:], mu[:], AF.Square)
            var = cst.tile([128, 1], F32, tag="var")
            nc.scalar.activation(var[:], gst[:, 1:2], AF.Copy, scale=1.0 / NTOT)
            nc.vector.tensor_sub(var[:], var[:], musq[:])
            sd = cst.tile([128, 1], F32, tag="sd")
            nc.scalar.activation(sd[:], var[:], AF.Sqrt,
                                 bias=biasb[:, 18:19])
            rstd = cst.tile([128, 1], F32, tag="rstd")
            nc.vector.reciprocal(rstd[:], sd[:])
            bn_scope.__exit__(None, None, None)

            # ======================= post-BN network =========================
            with (
                tc.tile_pool(name="ps_p", bufs=3, space="PSUM") as ps_p,
                tc.tile_pool(name="ps_l", bufs=2, space="PSUM") as ps_l,
                tc.tile_pool(name="ps_o", bufs=2, space="PSUM") as ps_o,
            ):
                post_scope = nc.named_scope("post")
                post_scope.__enter__()

                def leaky_from_psum(dst, src_ps, bias_ap):
                    tmp = wp2.tile([128, NB], F32, tag="tmp")
                    nc.scalar.activation(tmp[:], src_ps, AF.Identity,
                                         bias=bias_ap)
                    nc.vector.scalar_tensor_tensor(
                        out=dst, in0=tmp[:], scalar=0.01, in1=tmp[:],
                        op0=OP.mult, op1=OP.max)

                for g in range(NAG):
                    col = bass.ts(g, NB)
                    nc.vector.tensor_scalar(
                        out=inps[:, col], in0=inps[:, col],
                        scalar1=mu[:], scalar2=rstd[:],
                        op0=OP.subtract, op1=OP.mult)
                    pe_ = ps_p.tile([128, NB], F32, tag="pp")
                    nc.tensor.matmul(pe_, encw[:], inps[:, col],
                                     start=True, stop=True)
                    leaky_from_psum(saT[:, g, :], pe_[:], biasb[:, 5:6])
                    pk = ps_p.tile([128, NB], F32, tag="pp")
                    nc.tensor.matmul(pk, attw[:, 0, :], saT[:, g, :],
                                     start=True, stop=True)
                    nc.scalar.copy(keysT[:, g, :], pk[:])
                    pv = ps_p.tile([128, NB], F32, tag="pp")
                    nc.tensor.matmul(pv, attw[:, 1, :], saT[:, g, :],
                                     start=True, stop=True)
                    leaky_from_psum(valsT[:, g, :], pv[:], biasb[:, 6:7])

                for g in range(NAG):
                    col = bass.ts(g, NB)
                    oa, ob = [o for o in range(NAG) if o != g]
                    pq_ = ps_p.tile([128, NB], F32, tag="pp")
                    nc.tensor.matmul(pq_, attw[:, 2 + g, :], saT[:, g, :],
                                     start=True, stop=True)
                    qT = wp2.tile([128, NB], F32R, tag="qt")
                    nc.scalar.copy(qT[:], pq_[:])
                    # logit difference (l_a - l_b) accumulated in one PSUM
                    pl = ps_l.tile([8, NB], F32, tag="pl")
                    for o, hcols, st in ((oa, slice(0, 8), True),
                                         (ob, slice(8, 16), False)):
                        prod = wp2.tile([128, NB], F32R, tag="prod")
                        nc.vector.tensor_mul(prod[:], qT[:], keysT[:, o, :])
                        nc.tensor.matmul(pl, hsum[:, hcols], prod[:],
                                         start=st, stop=not st)
                    # wa = sigmoid((l_a - l_b)/4); other = vb + wa*(va - vb)
                    wa = wp2.tile([8, NB], F32R, tag="wa")
                    nc.scalar.activation(wa[:], pl[:], AF.Sigmoid, scale=0.25)
                    pw = ps_p.tile([128, NB], F32, tag="pp")
                    nc.tensor.matmul(pw, hbc[:], wa[:], start=True, stop=True)
                    dv = wp2.tile([128, NB], F32, tag="dv")
                    nc.vector.tensor_sub(dv[:], valsT[:, oa, :],
                                         valsT[:, ob, :])
                    m1 = wp2.tile([128, NB], F32, tag="m1")
                    nc.vector.tensor_mul(m1[:], pw[:], dv[:])
                    oth = wp2.tile([128, NB], F32R, tag="oth")
                    nc.vector.tensor_add(oth[:], m1[:], valsT[:, ob, :])

                    emb = (saT[:, g, :], oth[:])
                    for net in range(2):
                        h_prev = emb
                        for layer in range(2):
                            coff = 512 * layer
                            h_new = []
                            for mb in range(2):
                                pm = ps_p.tile([128, NB], F32, tag="pp")
                                for kb in range(2):
                                    c0 = coff + kb * 256 + mb * 128
                                    nc.tensor.matmul(
                                        pm, mlpw[:, net, c0:c0 + 128],
                                        h_prev[kb], start=(kb == 0),
                                        stop=(kb == 1))
                                hm = wp4.tile([128, NB], F32R, tag="h")
                                bcol = (7 if net == 0 else 12) + 2 * layer + mb
                                evict_relu(hm[:], pm[:],
                                           biasb[:, bcol:bcol + 1], mb == 0)
                                h_new.append(hm)
                            h_prev = h_new
                        po_ = ps_o.tile([1, NB], F32, tag="pout")
                        for kb in range(2):
                            nc.tensor.matmul(
                                po_,
                                mlpw[:, net, 1024 + 64 * kb:1025 + 64 * kb],
                                h_prev[kb][:], start=(kb == 0), stop=(kb == 1))
                        bcol = 11 if net == 0 else 16
                        dst = (outq1 if net == 0 else outq2)[0:1, col]
                        nc.scalar.activation(dst, po_[:], AF.Identity,
                                             bias=biasb[0:1, bcol:bcol + 1])
                post_scope.__exit__(None, None, None)
            nc.sync.dma_start(out=out_d[0:1, :], in_=outq1[:])
            nc.sync.dma_start(out=out_d[1:2, :], in_=outq2[:])
    return nc


# ======================= host-side preparation ===========================

def _prep_shared(i):
    f32 = np.float32
    w1 = np.asarray(i["conv_w1"], f32)[:, 0, :]           # [32, 5]
    w2 = np.asarray(i["conv_w2"], f32)                    # [32, 32, 3]
    fw1 = np.asarray(i["fc_w1"], f32)                     # [256, 8000]
    fw2 = np.asarray(i["fc_w2"], f32)                     # [10, 256]
    encw_ = np.asarray(i["enc_w"], f32)                   # [128, 60]
    Wk = np.asarray(i["Wk"], f32)
    Wv = np.asarray(i["Wv"], f32)
    Wq = np.asarray(i["Wq"], f32)

    t1 = np.zeros((128, 33, 128), f32)
    for idx in range(32):
        r0 = 4 * idx if idx < 31 else 124
        for dp in range(4):
            for j in range(5):
                r = r0 + dp + j
                if r < 128:
                    t1[r, idx, dp::4] = w1[:, j]
    for dp in range(4):
        for r in range(4):
            j = r + 4 - dp
            if 0 <= j < 5:
                t1[r, 32, dp::4] = w1[:, j]

    t2 = np.zeros((128, 320), f32)
    for dp in range(4):
        for j in range(3):
            e = dp + j
            for cp in range(32):
                if e < 4:
                    t2[4 * cp + e, dp:128:4] = w2[:, cp, j]
                else:
                    t2[4 * cp + (e - 4), 128 + dp:256:4] = w2[:, cp, j]
    for dp in range(2):
        for j in range(3):
            e = dp + j
            for cp in range(32):
                t2[4 * cp + e, 256 + dp:320:2] = w2[:, cp, j]

    fc1w = np.zeros((128, QT, 256), f32)
    for q in range(QT - 1):
        for dp in range(4):
            p = 4 * q + dp
            fc1w[dp::4, q, :] = fw1[:, p::P2].T
    for dp in range(2):
        fc1w[dp:64:2, QT - 1, :] = fw1[:, (248 + dp)::P2].T

    fc2w = np.zeros((128, 2, 16), f32)
    for kb in range(2):
        fc2w[:, kb, 0:OUTF] = fw2[:, 128 * kb:128 * kb + 128].T

    encw_full = np.zeros((128, 128), f32)
    encw_full[0:STATE, :] = encw_.T[0:STATE, :]            # obs rows
    encw_full[64:64 + OUTF, :] = encw_.T[50:60, :]         # feats rows
    encw_full[96:96 + ACTD, :] = encw_.T[48:50, :]         # acts rows

    attw = np.zeros((128, 5, 128), f32)
    attw[:, 0, :] = Wk.reshape(128, H).T
    attw[:, 1, :] = Wv.reshape(128, H).T
    for g in range(NAG):
        attw[:, 2 + g, :] = Wq[g].reshape(128, H).T

    hs = np.kron(np.eye(8, dtype=f32), np.ones((16, 1), f32))  # [128, 8]
    hsum = np.concatenate([hs, -hs], axis=1)                   # [128, 16]
    hbc = np.ascontiguousarray(hs.T)                           # [8, 128]

    mlpw = np.zeros((128, 2, 1152), f32)
    for net, pre in enumerate(("q1", "q2")):
        mw1 = np.asarray(i[pre + "_w1"], f32)
        mw2 = np.asarray(i[pre + "_w2"], f32)
        mw3 = np.asarray(i[pre + "_w3"], f32)
        for kb in range(2):
            mlpw[:, net, kb * 256:(kb + 1) * 256] = \
                mw1[:, 128 * kb:128 * kb + 128].T
            mlpw[:, net, 512 + kb * 256:512 + (kb + 1) * 256] = \
                mw2[:, 128 * kb:128 * kb + 128].T
            mlpw[:, net, 1024 + 64 * kb] = mw3[0, 128 * kb:128 * kb + 128]

    bias = np.zeros((128, 20), f32)
    bias[:, 0] = np.repeat(np.asarray(i["conv_b1"], f32), 4)
    bias[:, 1] = np.repeat(np.asarray(i["conv_b2"], f32), 4)
    bias[:, 2] = np.asarray(i["fc_b1"], f32)[0:128]
    bias[:, 3] = np.asarray(i["fc_b1"], f32)[128:256]
    bias[0:OUTF, 4] = np.asarray(i["fc_b2"], f32)
    bias[:, 5] = np.asarray(i["enc_b"], f32)
    bias[:, 6] = np.asarray(i["bv"], f32).reshape(128)
    for net, pre in enumerate(("q1", "q2")):
        b1 = np.asarray(i[pre + "_b1"], f32)
        b2 = np.asarray(i[pre + "_b2"], f32)
        b3 = np.asarray(i[pre + "_b3"], f32)
        c0 = 7 if net == 0 else 12
        bias[:, c0] = b1[0:128]
        bias[:, c0 + 1] = b1[128:256]
        bias[:, c0 + 2] = b2[0:128]
        bias[:, c0 + 3] = b2[128:256]
        bias[0, 11 if net == 0 else 16] = b3[0]
    bias[0:64, 17] = np.repeat(np.asarray(i["conv_b2"], f32), 2)
    bias[:, 18] = EPS

    return {
        "t1": t1, "t2": t2, "fc1w": fc1w, "fc2w": fc2w,
        "encw": encw_full, "attw": attw,
        "hsum": hsum, "hbc": hbc, "mlpw": mlpw, "bias": bias,
    }


def _shard(arr, c):
    out = np.empty((R, arr.shape[1]), np.float32)
    for g in range(NAG):
        out[g * BL:(g + 1) * BL] = arr[g * B + c * BL: g * B + (c + 1) * BL]
    return np.ascontiguousarray(out.T)


_CACHE = {}


def _get_prog():
    if "nc" not in _CACHE:
        nc = build_program()
        nc.finalize()
        _CACHE["nc"] = nc
    return _CACHE["nc"]


def _make_in_maps(inputs):
    shared = _prep_shared(inputs)
    obs = np.asarray(inputs["obs"], np.float32)
    acts = np.asarray(inputs["acts"], np.float32)
    scan = np.asarray(inputs["scan"], np.float32)
    in_maps = []
    for c in range(NCORES):
        m = dict(shared)
        m["zpad"] = np.zeros((30, R), np.float32)
        m["scan_t"] = _shard(scan, c)
        m["obs_t"] = _shard(obs, c)
        m["acts_t"] = _shard(acts, c)
        in_maps.append(m)
    return in_maps


def _gather(results):
    q = np.empty((2, NAG, B), np.float32)
    for c, r in enumerate(results):
        o = np.asarray(r["out"]).reshape(2, NAG, BL)
        q[:, :, c * BL:(c + 1) * BL] = o
    q1 = np.ascontiguousarray(q[0].reshape(NTOT, 1))
    q2 = np.ascontiguousarray(q[1].reshape(NTOT, 1))
    return q1, q2


def kernel(**inputs):
    nc = _get_prog()
    in_maps = _make_in_maps(inputs)
    if os.environ.get("KERNEL_BACKEND") == "sim":
        from concourse import bass_interp
        sim = bass_interp.MultiCoreSim(nc, NCORES)
        for c in range(NCORES):
            for k, v in in_maps[c].items():
                sim.cores[c].tensor(k)[:] = v
        sim.simulate()
        results = [{"out": np.array(sim.cores[c].tensor("out"))}
                   for c in range(NCORES)]
        return _gather(results)
    res = run_bass_kernel_spmd(nc, in_maps, list(range(NCORES)))
    return _gather(res.results)


def kernel_profiled(**inputs):
    """Like kernel() but NTFF-traced; returns ((q1, q2), exec_time_ns)."""
    _install_ntff_hook()
    nc = _get_prog()
    in_maps = _make_in_maps(inputs)
    res = run_bass_kernel_spmd(nc, in_maps, list(range(NCORES)), trace=True)
    return _gather(res.results), res.exec_time_ns


def _install_ntff_hook():
    """Provide antenv.axon_hooks (absent in this image) and register the
    ctypes NTFF profile hook against libaxon_pjrt.so."""
    import sys
    import types
    import ctypes
    import contextlib

    if "antenv.axon_hooks" not in sys.modules:
        mod = types.ModuleType("antenv.axon_hooks")
        mod._hook = None
        mod.set_axon_ntff_profile_hook = lambda h: setattr(mod, "_hook", h)
        mod.get_axon_ntff_profile_hook = lambda: mod._hook
        sys.modules["antenv.axon_hooks"] = mod
        import antenv
        antenv.axon_hooks = mod
    mod = sys.modules["antenv.axon_hooks"]
    if mod.get_axon_ntff_profile_hook() is not None:
        return
    so_path = "/opt/axon/libaxon_pjrt.so"
    lib = ctypes.CDLL(so_path)
    if not hasattr(lib, "axon_start_nrt_profile"):
        return
    lib.axon_start_nrt_profile.argtypes = [
        ctypes.POINTER(ctypes.c_int64), ctypes.c_size_t]
    lib.axon_start_nrt_profile.restype = ctypes.c_int64
    lib.axon_stop_nrt_profile.argtypes = [ctypes.c_char_p]
    lib.axon_stop_nrt_profile.restype = ctypes.c_int64

    @contextlib.contextmanager
    def _hook(output_dir, device_ids):
        import jax
        jax.devices()
        if device_ids:
            ids = (ctypes.c_int64 * len(device_ids))(*device_ids)
            rc = lib.axon_start_nrt_profile(ids, len(device_ids))
        else:
            rc = lib.axon_start_nrt_profile(None, 0)
        if rc != 0:
            raise RuntimeError(f"axon_start_nrt_profile rc={rc}")
        try:
            yield
        finally:
            n = lib.axon_stop_nrt_profile(str(output_dir).encode())
            if n < 0:
                raise RuntimeError(f"axon_stop_nrt_profile rc={n}")

    mod.set_axon_ntff_profile_hook(_hook)
